# revision 1
# baseline (speedup 1.0000x reference)
"""GQA causal self-attention (B=2, T=2048, C=2048, 16 Q heads / 4 KV heads,
head_dim=128) on 8 TRN2 NeuronCores.

Sharding: core = (batch b, kv-group g) for b in {0,1}, g in {0..3}.
Each core computes its batch's 4 Q heads that share KV head g, plus the
partial out-projection over those heads' rows of W_out. Host sums the 4
partials per batch and adds b_out.

Device layout choices (all feature-major, "T" on the free axis):
  - qT/kT [d=128 part, t free]  -> scores S^T[j,i] = kT_tile.T @ qT_slice
  - softmax over j (= partition axis of S^T): exp on ACT (scale=1/128
    fused), causal mask via gpsimd.affine_select, column-sum via
    ones-vector matmul, broadcast of 1/den back over partitions via a
    rank-1 (K=1) matmul.
  - y^T[d, i] = v_tile.T-free accumulation: lhsT = v[t,d] tiles, rhs = P^T.
  - out[t, e] = y^T as lhsT directly against W_out rows.
All matmul operands bf16 (fp32 PSUM accumulation); everything else fp32.
"""

import sys

if "/opt/trn_rl_repo" not in sys.path:
    sys.path.insert(0, "/opt/trn_rl_repo")

import numpy as np
import ml_dtypes

BF16 = ml_dtypes.bfloat16

B = 2
T = 2048
C = 2048
NH = 16
NKV = 4
D = 128
GQ = NH // NKV  # 4 q heads per kv head
N_CORES = 8
CC = C // 128  # 16 contraction chunks
TS = T // 512  # 4 t-slices
TT = T // 128  # 16 t-tiles
NF = GQ + 2  # feature chunks per core: 4 q heads + k + v

_CACHED = {}


def _build_bass(reps=1):
    import concourse.bass as bass
    import concourse.bacc as bacc
    import concourse.tile as tile
    import concourse.mybir as mybir

    bf = mybir.dt.bfloat16
    f32 = mybir.dt.float32
    Exp = mybir.ActivationFunctionType.Exp

    nc = bacc.Bacc(None, target_bir_lowering=False)

    # DRAM inputs (host pre-laid-out, see kernel())
    xT_d = nc.dram_tensor("xt", [128, CC, T], bf, kind="ExternalInput")
    wqkv_d = nc.dram_tensor("wqkv", [128, CC, NF * 128], bf, kind="ExternalInput")
    bqkv_d = nc.dram_tensor("bqkv", [128, NF], f32, kind="ExternalInput")
    cos_d = nc.dram_tensor("cosT", [128, T], f32, kind="ExternalInput")
    sin_d = nc.dram_tensor("sinT", [128, T], f32, kind="ExternalInput")
    swap_d = nc.dram_tensor("swp", [128, 128], bf, kind="ExternalInput")
    iden_d = nc.dram_tensor("idn", [128, 128], bf, kind="ExternalInput")
    wout_d = nc.dram_tensor("wout", [128, GQ, C], bf, kind="ExternalInput")
    out_d = nc.dram_tensor("out", [T, C], f32, kind="ExternalOutput")

    with tile.TileContext(nc) as tc:
        with (
            tc.tile_pool(name="persist", bufs=1) as pers,
            tc.tile_pool(name="xt", bufs=2) as xtp,
            tc.tile_pool(name="stage", bufs=3) as stg,
            tc.tile_pool(name="ptile", bufs=6) as ptp,
            tc.tile_pool(name="small", bufs=4) as smp,
            tc.tile_pool(name="osb", bufs=3) as osp,
            tc.tile_pool(name="ps_qkv", bufs=2, space="PSUM") as ppq,
            tc.tile_pool(name="ps_sc", bufs=2, space="PSUM") as pps,
            tc.tile_pool(name="ps_y", bufs=2, space="PSUM") as ppy,
            tc.tile_pool(name="ps_d", bufs=2, space="PSUM") as ppd,
        ):
            import contextlib
            loop_cm = tc.For_i(0, reps, 1) if reps > 1 else contextlib.nullcontext()
            with loop_cm:
                _body(nc, tc, mybir, bf, f32, Exp,
                      pers, xtp, stg, ptp, smp, osp, ppq, pps, ppy, ppd,
                      xT_d, wqkv_d, bqkv_d, cos_d, sin_d, swap_d, iden_d, wout_d, out_d)
    nc.compile()
    return nc


def _body(nc, tc, mybir, bf, f32, Exp,
          pers, xtp, stg, ptp, smp, osp, ppq, pps, ppy, ppd,
          xT_d, wqkv_d, bqkv_d, cos_d, sin_d, swap_d, iden_d, wout_d, out_d):
            # ---- persistent loads ----
            wq_sb = pers.tile([128, CC, NF * 128], bf)
            nc.sync.dma_start(wq_sb[:, 0:1, :], wqkv_d[:, 0:1, :])
            xt0 = xtp.tile([128, CC, 512], bf, tag="xt")
            nc.sync.dma_start(xt0[:, 0:1, :], xT_d[:, 0:1, 0:512])
            nc.sync.dma_start(wq_sb[:, 1:4, :], wqkv_d[:, 1:4, :])
            for xc in range(1, 4):
                nc.sync.dma_start(xt0[:, xc * 4 - 3 : xc * 4 + 1, :],
                                  xT_d[:, xc * 4 - 3 : xc * 4 + 1, 0:512])
            nc.sync.dma_start(xt0[:, 13:16, :], xT_d[:, 13:16, 0:512])
            bq_sb = pers.tile([128, NF], f32)
            nc.sync.dma_start(bq_sb[:], bqkv_d[:])
            swap_sb = pers.tile([128, 128], bf)
            nc.sync.dma_start(swap_sb[:], swap_d[:])
            iden_sb = pers.tile([128, 128], bf)
            nc.sync.dma_start(iden_sb[:], iden_d[:])
            for wc in range(1, 4):
                nc.sync.dma_start(wq_sb[:, wc * 4 : (wc + 1) * 4, :],
                                  wqkv_d[:, wc * 4 : (wc + 1) * 4, :])
            cos_sb = pers.tile([128, T], f32)
            nc.sync.dma_start(cos_sb[:], cos_d[:])
            sin_sb = pers.tile([128, T], f32)
            nc.sync.dma_start(sin_sb[:], sin_d[:])
            wout_sb = pers.tile([128, GQ, C], bf)
            nc.sync.dma_start(wout_sb[:], wout_d[:])
            ones_sb = pers.tile([128, 1], bf)
            nc.vector.memset(ones_sb[:], 1.0)

            # persistent activations
            qk_sb = pers.tile([128, GQ + 1, T], bf)  # rotated q0..q3, k
            v_sb = pers.tile([128, TT, 128], bf)  # v in [t-part, d] tiles
            y_sb = pers.tile([128, GQ, T], bf)  # y^T per head

            mul = mybir.AluOpType.mult
            add = mybir.AluOpType.add

            # ---- phase 1: QKV + RoPE + v transpose ----
            for ts in range(TS):
                tsl = slice(ts * 512, (ts + 1) * 512)
                if ts == 0:
                    xt = xt0
                else:
                    xt = xtp.tile([128, CC, 512], bf, tag="xt")
                    nc.sync.dma_start(xt[:], xT_d[:, :, tsl])
                for f in range(NF):
                    ps = ppq.tile([128, 512], f32, tag="qkvps")
                    for cc in range(CC):
                        nc.tensor.matmul(
                            ps[:],
                            wq_sb[:, cc, f * 128 : (f + 1) * 128],
                            xt[:, cc, :],
                            start=(cc == 0),
                            stop=(cc == CC - 1),
                        )
                    # bias add (also PSUM->SBUF move), bf16 out
                    raw = stg.tile([128, 512], bf, tag="raw")
                    nc.vector.tensor_tensor(
                        raw[:], ps[:], bq_sb[:, f : f + 1].to_broadcast((128, 512)), add
                    )
                    if f < NF - 1:
                        # rope: rot = raw*cos + swap(raw)*sinsign
                        psw = pps.tile([128, 512], f32, tag="sps")
                        nc.tensor.matmul(psw[:], swap_sb[:], raw[:], start=True, stop=True)
                        tmp = stg.tile([128, 512], bf, tag="ropetmp")
                        nc.vector.tensor_tensor(tmp[:], psw[:], sin_sb[:, tsl], mul)
                        nc.vector.tensor_tensor(
                            qk_sb[:, f, tsl], raw[:], cos_sb[:, tsl], mul
                        )
                        nc.vector.tensor_tensor(
                            qk_sb[:, f, tsl], qk_sb[:, f, tsl], tmp[:], add
                        )
                    else:
                        # v: transpose [d, t] -> [t, d] via PE
                        for k in range(4):
                            pst = pps.tile([128, 128], bf, tag="sps")
                            nc.tensor.transpose(
                                pst[:], raw[:, k * 128 : (k + 1) * 128], iden_sb[:]
                            )
                            nc.any.tensor_copy(v_sb[:, ts * 4 + k, :], pst[:])

            # ---- phase 2+3: per i-slice: attention (4 heads) then out-proj ----
            for s in range(TS):
                isl = slice(s * 512, (s + 1) * 512)
                njt = 4 * (s + 1)
                for h in range(GQ):
                    psy = ppy.tile([128, 512], f32, tag="yps")
                    psd = ppd.tile([1, 512], f32, tag="dps")
                    for jt in range(njt):
                        # columns i < 128*jt are fully masked: skip them
                        off = max(0, 128 * jt - 512 * s)
                        pss = pps.tile([128, 512], f32, tag="sps")
                        nc.tensor.matmul(
                            pss[:, off:512],
                            qk_sb[:, GQ, jt * 128 : (jt + 1) * 128],
                            qk_sb[:, h, s * 512 + off : (s + 1) * 512],
                            start=True,
                            stop=True,
                        )
                        P = ptp.tile([128, 512], bf, tag="P")
                        nc.scalar.activation(
                            P[:, off:512], pss[:, off:512], Exp, scale=1.0 / 128.0
                        )
                        if jt >= 4 * s:
                            # triangular block: keep where p <= y (y rel. to off)
                            nc.gpsimd.affine_select(
                                out=P[:, off : off + 128],
                                in_=P[:, off : off + 128],
                                pattern=[[1, 128]],
                                compare_op=mybir.AluOpType.is_ge,
                                fill=0.0,
                                base=0,
                                channel_multiplier=-1,
                            )
                        nc.tensor.matmul(
                            psy[:, off:512],
                            v_sb[:, jt, :],
                            P[:, off:512],
                            start=(jt == 0),
                            stop=(jt == njt - 1),
                        )
                        nc.tensor.matmul(
                            psd[:, off:512],
                            ones_sb[:],
                            P[:, off:512],
                            start=(jt == 0),
                            stop=(jt == njt - 1),
                        )
                    rden = smp.tile([1, 512], f32, tag="rden")
                    nc.vector.reciprocal(rden[:], psd[:])
                    rdb = smp.tile([128, 512], f32, tag="rdb")
                    nc.gpsimd.partition_broadcast(rdb[:], rden[:])
                    nc.vector.tensor_tensor(y_sb[:, h, isl], psy[:], rdb[:], mul)

                for tt in range(4 * s, 4 * s + 4):
                    o_sb = osp.tile([128, C], f32, tag="osb")
                    for es in range(4):
                        pso = ppy.tile([128, 512], f32, tag="yps")
                        for h in range(GQ):
                            nc.tensor.matmul(
                                pso[:],
                                y_sb[:, h, tt * 128 : (tt + 1) * 128],
                                wout_sb[:, h, es * 512 : (es + 1) * 512],
                                start=(h == 0),
                                stop=(h == GQ - 1),
                            )
                        if es % 2 == 0:
                            nc.vector.tensor_copy(
                                o_sb[:, es * 512 : (es + 1) * 512], pso[:]
                            )
                        else:
                            nc.scalar.copy(
                                o_sb[:, es * 512 : (es + 1) * 512], pso[:]
                            )
                    nc.sync.dma_start(out_d[tt * 128 : (tt + 1) * 128, :], o_sb[:])


def _host_prep(x, rope_cache, W_qkv, b_qkv, W_out):
    """Build the 8 per-core input dicts."""
    q_dim = NH * D  # 2048
    kv_dim = NKV * D  # 512

    # rope tables in [d, t] layout
    sin = rope_cache[:, 0::2].astype(np.float32)  # [T, 64]
    cos = rope_cache[:, 1::2].astype(np.float32)
    cos2T = np.empty((128, T), np.float32)
    sinsT = np.empty((128, T), np.float32)
    cos2T[0::2] = cos.T
    cos2T[1::2] = cos.T
    sinsT[0::2] = -sin.T
    sinsT[1::2] = sin.T

    swap = np.zeros((128, 128), BF16)
    idx = np.arange(128)
    swap[idx, idx ^ 1] = 1
    iden = np.eye(128, dtype=BF16)

    in_maps = []
    for b in range(B):
        xT = np.ascontiguousarray(x[b].T.astype(BF16))  # [C, T]
        xT = xT.reshape(CC, 128, T).transpose(1, 0, 2)  # [128, CC, T]
        xT = np.ascontiguousarray(xT)
        for g in range(NKV):
            cols = np.concatenate(
                [
                    np.arange(4 * g * D, (4 * g + 4) * D),  # 4 q heads
                    np.arange(q_dim + g * D, q_dim + (g + 1) * D),  # k head
                    np.arange(q_dim + kv_dim + g * D, q_dim + kv_dim + (g + 1) * D),
                ]
            )
            wq = W_qkv[:, cols].astype(BF16)  # [C, 768]
            wq = np.ascontiguousarray(
                wq.reshape(CC, 128, NF * 128).transpose(1, 0, 2)
            )  # [128, CC, 768]
            bq = np.ascontiguousarray(
                b_qkv[cols].astype(np.float32).reshape(NF, 128).T
            )  # [128, NF]
            wo = W_out[4 * g * D : (4 * g + 4) * D, :].astype(BF16)  # [512, C]
            wo = np.ascontiguousarray(
                wo.reshape(GQ, 128, C).transpose(1, 0, 2)
            )  # [128, GQ, C]
            in_maps.append(
                {
                    "xt": xT,
                    "wqkv": wq,
                    "bqkv": bq,
                    "cosT": cos2T,
                    "sinT": sinsT,
                    "swp": swap,
                    "idn": iden,
                    "wout": wo,
                }
            )
    return in_maps


def kernel(x, rope_cache, W_qkv, b_qkv, W_out, b_out, _trace=False):
    from concourse.bass_utils import run_bass_kernel_spmd

    if "nc" not in _CACHED:
        _CACHED["nc"] = _build_bass()
    nc = _CACHED["nc"]

    in_maps = _host_prep(
        np.asarray(x), np.asarray(rope_cache), np.asarray(W_qkv),
        np.asarray(b_qkv), np.asarray(W_out),
    )
    res = run_bass_kernel_spmd(nc, in_maps, core_ids=list(range(N_CORES)), trace=_trace)
    _CACHED["last_result"] = res

    out = np.zeros((B, T, C), np.float32)
    for b in range(B):
        acc = res.results[b * NKV]["out"].astype(np.float32)
        for g in range(1, NKV):
            acc = acc + res.results[b * NKV + g]["out"]
        out[b] = acc + np.asarray(b_out)[None, :]
    return out

